# revision 6
# baseline (speedup 1.0000x reference)
"""CombinedAttentionProcessor kernel for 8 Trainium2 NeuronCores (fp8).

Problem: B=2, S=4096, C=640, H=8 heads, D=80 head_dim.
    q/k/v = hs @ W{q,k,v}.T ; per-(b,h): softmax(q k^T / sqrt(D)) v ;
    out = attn @ Wo.T + bo + residual.

Sharding: 16 (batch, head) groups -> 2 per core. Each core computes its 2
heads' attention and a partial output projection [S, C]; the host sums the
4 partials per batch, descales by 1/256, and adds bias + residual.

Matmuls run in fp8e4m3 DoubleRow perf mode (2 contraction subtiles per
instruction, 0.5 cycles/row): weights host-scaled by 16 (fp8 dynamic
range). The softmax exp is split across ACT and DVE per key-tile pair
(GPSIMD cannot access PSUM): ACT computes exp natively; DVE computes a
Schraudolph exp: uint8 = round(score*8*log2e*scale) bitcast to fp8e4m3
(float->uint8 saturates at 0 on HW, clamping the low tail). A constant
contraction row (qT/kT partition 40, value 48 -> +2304 in every psum
score) centers both paths on p~ = exp(s - 3.85), keeping the fp8 pt in
range for row-max scores up to ~9.3 (real data reaches ~8); the shared
bias cancels in the softmax normalization.

Phases: A projects q/k/v for both heads through a 4-slot PSUM ring; B is
one flat jg-stream over all 16 (head, chunk) softmaxes (PE emits score
pairs, ACT/DVE exponentiate, AV DoubleRow matmuls trail by AVLAG with the
ones-column at 96 giving the row sums); C runs the output projection.

Hardware rules learned the hard way (violations = NaN or dead device):
  - PE matmuls honor only ONE semaphore wait: tiny observer matmuls make
    PE see every DMA-queue semaphore once; engine assignment keeps each
    real matmul's remaining unobserved waits on a single semaphore.
  - GPSIMD cannot access PSUM (memsets only).
  - dual-fp8 Ldweights: subtile column count % 4 == 0 and subtile byte
    stride % 16 == 0 (hence VS=104), and 64 < rows < 128 is invalid
    (hence the non-DoubleRow output projection with 80 rows).
  - fp8 overflow (>240) produces inf/NaN, not saturation.
"""
import sys

if "/opt/trn_rl_repo" not in sys.path:
    sys.path.insert(0, "/opt/trn_rl_repo")

import numpy as np

B, S, C = 2, 4096, 640
H, D = 8, 80
HPC = 2          # heads per core
NCORES = 8
KC = 5           # real contraction tiles over C
KC6 = 6          # padded to even for DoubleRow pairs
NCB = 8          # hsT DMA column batches
WSCALE = 16.0    # host weight scale (fp8 dynamic range)
SCALE = 1.0 / float(np.sqrt(D))
SEFF = SCALE / (WSCALE * WSCALE)        # psum score -> true scaled score
SCHA = SEFF * 8.0 / float(np.log(2.0))  # Schraudolph slope (fp8e4m3, m=3)
# Schraudolph bias is embedded in the scores via a constant contraction row
# (qT/kT row 40, half 0, value 48.0 each -> +2304 in every psum score), so
# the uint8 cast input is >= 0 (no negative wrap; low tail clamps via max).
BROW = 48.0
BPSUM = BROW * BROW                     # 9216
# ACT path must encode the same value: exp(seff*psum + EBIAS) == 2^((i-56)/8)
EBIAS = float(-BPSUM * SEFF - (56.0 - BPSUM * SCHA) * np.log(2.0) / 8.0)
VS = 104  # dual-fp8 ldweights: cols % 4 == 0, subtile stride % 16 == 0
ONESCOL = 96

_NC_CACHE = {}


def build_nc(s=S):
    import concourse.bacc as bacc
    import concourse.mybir as mybir
    import concourse.tile as tile
    from concourse.tile import add_dep_helper

    f32 = mybir.dt.float32
    bf16 = mybir.dt.bfloat16
    fp8 = mybir.dt.float8e4
    u8 = mybir.dt.uint8
    DR = mybir.MatmulPerfMode.DoubleRow
    Exp = mybir.ActivationFunctionType.Exp

    njt = s // 128    # key tiles
    nit = s // 128    # output i-tiles
    nch = s // 512    # query chunks
    njg = njt // 2    # key-tile pairs per chunk
    cbw = s // NCB    # hsT column batch width
    assert s % 512 == 0 and njt % 4 == 0

    nc = bacc.Bacc("TRN2", target_bir_lowering=False, debug=False,
                   num_devices=NCORES)

    hsT = nc.dram_tensor("hsT", [128, NCB * KC6 * cbw], fp8,
                         kind="ExternalInput")
    wq = nc.dram_tensor("wq", [128, KC6 * HPC * D], fp8, kind="ExternalInput")
    wk = nc.dram_tensor("wk", [128, KC6 * HPC * D], fp8, kind="ExternalInput")
    wv = nc.dram_tensor("wv", [128, KC6 * HPC * D], fp8, kind="ExternalInput")
    wo = nc.dram_tensor("wo", [128, HPC * C], fp8, kind="ExternalInput")
    qkb = nc.dram_tensor("qkb", [1, 2 * s], fp8, kind="ExternalInput")
    o_dram = nc.dram_tensor("o", [128, nit * C], bf16,
                             kind="ExternalOutput")

    # engine-assignment helpers --------------------------------------------
    # exp halves per chunk: proportional-rate greedy schedule so each
    # engine's exp time per chunk is equal (ACT 612ns, DVE 658, Pool 806)
    # GPSIMD cannot access PSUM -> only ACT and DVE can read scores.
    _counts = {"A": 9, "D": 7}
    _cost = {"A": 1038.0, "D": 1192.0}
    _n16 = sum(_counts.values())
    EXP_PAT16 = []
    _load = {k: 0.0 for k in _counts}
    for _i in range(_n16):
        pick = max(_counts,
                   key=lambda k: (_i + 1) * _counts[k] / _n16
                   - _load[k] / _cost[k])
        _load[pick] += _cost[pick]
        EXP_PAT16.append(pick)
    if EXP_PAT16[0] != "D":
        EXP_PAT16[EXP_PAT16.index("D")] = EXP_PAT16[0]
        EXP_PAT16[0] = "D"
    cp_state = {"i": 0}

    with tile.TileContext(nc) as tc:
        with (
            tc.tile_pool(name="persist", bufs=1) as pp,
            tc.tile_pool(name="ppt", bufs=8) as ppt,
            tc.tile_pool(name="pbcs", bufs=2) as pbcs,
            tc.tile_pool(name="pobuf", bufs=2) as pobuf,
            tc.tile_pool(name="psc_ps", bufs=3, space="PSUM") as psc,
            tc.tile_pool(name="pprj_ps", bufs=1, space="PSUM") as pprj,
            tc.tile_pool(name="pav_ps", bufs=1, space="PSUM") as pav,
        ):
            # ---- persistent tiles ----
            hsT_sb = pp.tile([128, KC6, s], fp8, name="hsT_sb")
            wq_sb = pp.tile([128, KC6, HPC, 2, 40], fp8, name="wq_sb")
            wk_sb = pp.tile([128, KC6, HPC, 2, 40], fp8, name="wk_sb")
            wv_sb = pp.tile([128, KC6, HPC * D], fp8, name="wv_sb")
            wo_sb = pp.tile([128, HPC, C], fp8, name="wo_sb")
            qT = [pp.tile([128, 2, s], fp8, name=f"qT{h}") for h in range(HPC)]
            kT = [pp.tile([128, 2, s], fp8, name=f"kT{h}") for h in range(HPC)]
            v_sb = pp.tile([128, njt, HPC, VS], fp8, name="v_sb")
            avn = pp.tile([128, HPC, s], fp8, name="avn")
            recip_sb = pp.tile([128, 512], bf16, name="recip_sb")
            ones_sb = pp.tile([128, D], bf16, name="ones_sb")
            ebias = pp.tile([128, 1], f32, name="ebias")

            nc.vector.memset(ebias[:], EBIAS)
            nc.gpsimd.memset(recip_sb[:, :], 0.0)
            nc.gpsimd.memset(ones_sb[:, :], 0.0)
            nc.gpsimd.memset(ones_sb[0:1, :], 1.0)
            # v data cols 0:80 come from the projection; only the pad and the
            # denominator ones-column need initialization
            nc.gpsimd.memset(v_sb[:, :, :, D:VS], 0.0)
            nc.gpsimd.memset(v_sb[:, :, :, ONESCOL], 1.0)
            # ---- input DMAs (cb0 first so projections start early) ----
            def dma_cb(cb):
                nc.sync.dma_start(
                    hsT_sb[:, :, cb * cbw:(cb + 1) * cbw],
                    hsT[:, cb * KC6 * cbw:(cb + 1) * KC6 * cbw])

            dma_cb(0)
            nc.sync.dma_start(wk_sb.rearrange("p a b c d -> p (a b c d)"),
                              wk[:, :])
            nc.sync.dma_start(wv_sb.rearrange("p a b -> p (a b)"), wv[:, :])
            nc.sync.dma_start(wq_sb.rearrange("p a b c d -> p (a b c d)"),
                              wq[:, :])
            dma_cb(1)
            dma_cb(2)
            # softmax bias row at partition 40 (via DMA: engines can't
            # start an AP at a non-32-aligned partition)
            qkb_dmas = [nc.sync.dma_start(t[40:41, 0:2, :], qkb[:, :])
                        for t in qT + kT]
            for cb in range(3, NCB):
                dma_cb(cb)
            nc.sync.dma_start(wo_sb.rearrange("p a b -> p (a b)"), wo[:, :])

            def copy_eng(which=None):
                """Rotate copies across engines for balance."""
                if which == "A":
                    return nc.scalar
                if which == "D":
                    return nc.vector
                if which == "P":
                    return nc.gpsimd
                i = cp_state["i"] = cp_state["i"] + 1
                return (nc.scalar, nc.vector)[i % 2]

            def ecopy(eng, dst, src):
                if eng is nc.scalar:
                    eng.copy(dst, src)
                else:
                    eng.tensor_copy(dst, src)

            # ---- projection units (all-fp8 DoubleRow) ----
            prj_state = {"i": 0}

            def pe_observe(src_ap, extra_dep=None):
                """Tiny matmul so PE observes the semaphore guarding
                src_ap (PE matmuls only honor a single sync wait)."""
                dum = pprj.tile([8, 8], f32, name="dum", tag="prj")
                mm = nc.tensor.matmul(dum[:], src_ap, src_ap, start=True,
                                      stop=True, skip_group_check=True)
                if extra_dep is not None:
                    add_dep_helper(mm.ins, extra_dep.ins,
                                   reason="observe DMA sem on PE")

            def prj_tile(shape):
                i = prj_state["i"] = prj_state["i"] + 1
                if i % 4 == 3:
                    return pprj.tile(shape, f32, name="prj_ps", tag="prj")
                return psc.tile(shape, f32, name="prj_ps", tag="scslot")

            def emit_qk_chunk(h, w_sb, dst, iq, ceng=None):
                """dst[0:40, 0:2, iq*512:(iq+1)*512] = head-h projection."""
                i0 = iq * 512
                for half in range(2):
                    ps = prj_tile([40, 512])
                    for p in range(KC6 // 2):
                        nc.tensor.matmul(
                            ps[:],
                            w_sb[:, 2 * p:2 * p + 2, h, half, :],
                            hsT_sb[:, 2 * p:2 * p + 2, i0:i0 + 512],
                            start=(p == 0), stop=(p == KC6 // 2 - 1),
                            perf_mode=DR,
                        )
                    ecopy(copy_eng(ceng),
                          dst[0:40, half, i0:i0 + 512], ps[:])

            def emit_v_tile(jt, ceng=None):
                ps = prj_tile([128, HPC, D])
                for p in range(KC6 // 2):
                    nc.tensor.matmul(
                        ps.rearrange("p a b -> p (a b)"),
                        hsT_sb[:, 2 * p:2 * p + 2, jt * 128:(jt + 1) * 128],
                        wv_sb[:, 2 * p:2 * p + 2, :],
                        start=(p == 0), stop=(p == KC6 // 2 - 1),
                        perf_mode=DR,
                    )
                ecopy(copy_eng(ceng), v_sb[:, jt, 0:2, 0:D], ps[:, :, :])

            # ---- attention: flat jg stream across all chunks ----
            # (no per-chunk pipeline drain: exp engines stay fed across
            # chunk boundaries; AV matmuls trail by AVLAG positions)
            AVLAG = 4
            av_state = {}

            def emit_norm(h, i8):
                """Normalize chunk (h, i8): avn = av[0:D] / av[96]."""
                i0 = i8 * 512
                av = av_state.pop((h, i8))
                with nc.allow_low_precision(
                        reason="bf16 recip feeds broadcast matmul"):
                    nc.vector.reciprocal(recip_sb[0:1, 0:512],
                                         av[ONESCOL:ONESCOL + 1, :])
                av2 = pbcs.tile([D, 512], f32, name="av2")
                nc.vector.tensor_copy(av2[:], av[0:D, :])
                bc = pprj.tile([D, 512], f32, name="bc_ps", tag="prj")
                nc.tensor.matmul(bc[:], ones_sb[:], recip_sb[:, 0:512],
                                 start=True, stop=True)
                nc.vector.tensor_mul(avn[0:D, h, i0:i0 + 512], av2[:],
                                      bc[:])

            def attention_stream(chunks, filler_hook=None):
                pend = []
                n = len(chunks)
                for g in range(n * njg + AVLAG):
                    if g < n * njg:
                        h, i8 = chunks[g // njg]
                        jg = g % njg
                        i0 = i8 * 512
                        if filler_hook is not None and jg % 4 == 3:
                            filler_hook()
                        pt = ppt.tile([128, 2, 512], fp8, name="pt")
                        sc = psc.tile([128, 2, 512], f32, name="sc_ps",
                                      tag="scslot")
                        for jj in range(2):
                            j = 2 * jg + jj
                            nc.tensor.matmul(
                                sc[:, jj, :],
                                kT[h][0:41, 0:2, j * 128:(j + 1) * 128],
                                qT[h][0:41, 0:2, i0:i0 + 512],
                                start=True, stop=True,
                                perf_mode=DR,
                            )
                        if EXP_PAT16[g % 16] == "A":
                            nc.scalar.activation(
                                out=pt.rearrange("p a b -> p (a b)"),
                                in_=sc.rearrange("p a b -> p (a b)"),
                                func=Exp, scale=SEFF, bias=ebias[:],
                            )
                        else:
                            nc.vector.tensor_scalar(
                                pt.rearrange("p a b -> p (a b)").bitcast(u8),
                                sc.rearrange("p a b -> p (a b)"),
                                SCHA, 0.0,
                                op0=mybir.AluOpType.mult,
                                op1=mybir.AluOpType.max,
                            )
                        pend.append((h, i8, jg, pt))
                    if g >= AVLAG:
                        h2, i82, jg2, pt2 = pend.pop(0)
                        if jg2 == 0:
                            av_state[(h2, i82)] = pav.tile(
                                [VS, 512], f32, name="av_ps")
                        nc.tensor.matmul(
                            av_state[(h2, i82)][:],
                            v_sb[:, 2 * jg2:2 * jg2 + 2, h2, 0:VS],
                            pt2[:, :, :],
                            start=(jg2 == 0), stop=(jg2 == njg - 1),
                            perf_mode=DR,
                        )
                        if jg2 == njg - 1:
                            emit_norm(h2, i82)

            # ---- output projection ----
            o_state = {"buf": None}

            def emit_c_tile(g, ceng=None):
                if g % 4 == 0:
                    o_state["buf"] = pobuf.tile([128, 4, C], bf16,
                                                name="o_buf")
                o_buf = o_state["buf"]
                t0 = g * 128
                o_ps = psc.tile([128, C], f32, name="o_ps", tag="scslot")
                for n0, n1 in ((0, 512), (512, C)):
                    for hh in range(HPC):
                        nc.tensor.matmul(
                            o_ps[:, n0:n1], avn[0:D, hh, t0:t0 + 128],
                            wo_sb[0:D, hh, n0:n1],
                            start=(hh == 0), stop=(hh == HPC - 1),
                        )
                ecopy(copy_eng(ceng), o_buf[:, g % 4, :], o_ps[:])
                if g % 4 == 3:
                    nc.sync.dma_start(
                        o_dram[:, (g - 3) * C:(g + 1) * C],
                        o_buf.rearrange("p a b -> p (a b)"))

            # ============ Phase A: all projections ========================
            # PE must observe every input-DMA semaphore once (single-wait
            # rule) before real matmuls depend on them
            seen_cb = set()

            def observe_cb(cb):
                if cb not in seen_cb:
                    seen_cb.add(cb)
                    pe_observe(hsT_sb[0:8, 0, cb * cbw:cb * cbw + 8])

            pe_observe(wk_sb[0:8, 0, 0, 0, 0:8])
            pe_observe(wv_sb[0:8, 0, 0:8])
            pe_observe(wq_sb[0:8, 0, 0, 0, 0:8])
            observe_cb(0)

            def emit_qk_all(h, w_sb, dst, iq):
                for cb in range((iq * 512) // cbw,
                                ((iq + 1) * 512 - 1) // cbw + 1):
                    observe_cb(cb)
                emit_qk_chunk(h, w_sb, dst, iq)

            for iq in range(nch):
                emit_qk_all(0, wk_sb, kT[0], iq)
            for jt in range(njt):
                observe_cb((jt * 128) // cbw)
                emit_v_tile(jt)
            for iq in range(nch):
                emit_qk_all(0, wq_sb, qT[0], iq)
            for iq in range(nch):
                emit_qk_all(1, wk_sb, kT[1], iq)
            for iq in range(nch):
                emit_qk_all(1, wq_sb, qT[1], iq)
            # observe qkb bias rows and the tail projection copies on both
            # engines before attention consumes them
            for dma in qkb_dmas:
                pe_observe(ones_sb[0:8, 0:8], extra_dep=dma)
            pe_observe(qT[1][0:8, 1, s - 8:s])
            pe_observe(qT[1][0:8, 0, s - 8:s])
            pe_observe(kT[1][0:8, 1, s - 8:s])
            pe_observe(kT[1][0:8, 0, s - 8:s])
            pe_observe(qT[0][0:8, 1, s - 8:s])
            pe_observe(kT[0][0:8, 1, s - 8:s])
            pe_observe(v_sb[0:8, njt - 1, 1, 0:8])

            # ============ Phase B: attention (both heads) ==================
            attention_stream([(h, i8) for h in range(HPC)
                              for i8 in range(nch)])

            # ============ Phase C: out-projection ==========================
            pe_observe(wo_sb[0:8, 0, 0:8])
            pe_observe(avn[0:8, 1, s - 8:s])
            for g in range(nit):
                emit_c_tile(g, "D")

    nc.compile()
    return nc


def _get_nc(s=S):
    if s not in _NC_CACHE:
        _NC_CACHE[s] = build_nc(s)
    return _NC_CACHE[s]


def make_in_maps(hidden_states, Wq, Wk, Wv, Wo, s=S):
    """Shard full inputs into 8 per-core fp8 input dicts."""
    import ml_dtypes
    fp8 = ml_dtypes.float8_e4m3

    cbw = s // NCB
    hs = np.asarray(hidden_states, dtype=np.float32)
    Wq = np.asarray(Wq, dtype=np.float32)
    Wk = np.asarray(Wk, dtype=np.float32)
    Wv = np.asarray(Wv, dtype=np.float32)
    Wo = np.asarray(Wo, dtype=np.float32)

    # hsT[p, cb, kc, u] = hs[b][cb*cbw+u, kc*128+p]; kc=5 zero
    hsTs = []
    for b in range(B):
        t = hs[b].T.reshape(KC, 128, NCB, cbw)  # [kc, p, cb, u]
        hp8 = np.zeros((128, NCB, KC6, cbw), np.float32)
        hp8[:, :, :KC, :] = t.transpose(1, 2, 0, 3)
        hsTs.append(hp8.reshape(128, NCB * KC6 * cbw).astype(fp8))

    def pack_qk(W, hp):
        # -> [128, KC6, HPC, 2, 40]
        out = np.zeros((128, KC6, HPC, 2, 40), np.float32)
        rows = W[HPC * D * hp:HPC * D * (hp + 1), :] * WSCALE  # [160, C]
        r = rows.reshape(HPC, 2, 40, KC, 128)
        out[:, :KC] = r.transpose(4, 3, 0, 1, 2)
        return np.ascontiguousarray(
            out.reshape(128, KC6 * HPC * D)).astype(fp8)

    def pack_v(W, hp):
        out = np.zeros((128, KC6, HPC * D), np.float32)
        rows = W[HPC * D * hp:HPC * D * (hp + 1), :] * WSCALE  # [160, C]
        r = rows.reshape(HPC * D, KC, 128)
        out[:, :KC] = r.transpose(2, 1, 0)
        return np.ascontiguousarray(out.reshape(128, KC6 * HPC * D)).astype(fp8)

    def pack_wo(W, hp):
        # wo[p(d), h, c] = 16*Wo[c, hp*160 + h*80 + p]
        out = np.zeros((128, HPC, C), np.float32)
        cols = W[:, HPC * D * hp:HPC * D * (hp + 1)] * WSCALE  # [C, 160]
        out[0:D] = cols.T.reshape(HPC, D, C).transpose(1, 0, 2)
        return np.ascontiguousarray(out.reshape(128, HPC * C)).astype(fp8)

    in_maps = []
    for c in range(NCORES):
        b, hp = divmod(c, NCORES // B)
        qkb = np.zeros((1, 2 * s), np.float32)
        qkb[0, :s] = BROW
        in_maps.append({
            "hsT": hsTs[b],
            "qkb": qkb.astype(fp8),
            "wq": pack_qk(Wq, hp),
            "wk": pack_qk(Wk, hp),
            "wv": pack_v(Wv, hp),
            "wo": pack_wo(Wo, hp),
        })
    return in_maps


def unpermute_o(o_core, s=S):
    """[128, (s/128)*C] partition-major bf16 -> [s, C] f32."""
    nit = s // 128
    return np.asarray(o_core, dtype=np.float32).reshape(
        128, nit, C).transpose(1, 0, 2).reshape(s, C)


def assemble(results, hidden_states, bo, s=S):
    hs = np.asarray(hidden_states, dtype=np.float32)
    bo = np.asarray(bo, dtype=np.float32)
    out = np.empty((B, s, C), dtype=np.float32)
    ncb = NCORES // B
    descale = 1.0 / (WSCALE * WSCALE)
    for b in range(B):
        acc = unpermute_o(results[b * ncb]["o"], s).astype(np.float64)
        for k in range(1, ncb):
            acc = acc + unpermute_o(results[b * ncb + k]["o"], s)
        out[b] = (acc * descale + bo[None, :]).astype(np.float32) + hs[b]
    return out


def kernel(hidden_states, Wq, Wk, Wv, Wo, bo):
    from concourse.bass_utils import run_bass_kernel_spmd

    nc = _get_nc(S)
    in_maps = make_in_maps(hidden_states, Wq, Wk, Wv, Wo)
    res = run_bass_kernel_spmd(nc, in_maps, core_ids=list(range(NCORES)))
    return assemble(res.results, hidden_states, bo)


# revision 7
# speedup vs baseline: 1.0156x; 1.0156x over previous
"""CombinedAttentionProcessor kernel for 8 Trainium2 NeuronCores (fp8).

Problem: B=2, S=4096, C=640, H=8 heads, D=80 head_dim.
    q/k/v = hs @ W{q,k,v}.T ; per-(b,h): softmax(q k^T / sqrt(D)) v ;
    out = attn @ Wo.T + bo + residual.

Sharding: 16 (batch, head) groups -> 2 per core. Each core computes its 2
heads' attention and a partial output projection [S, C]; the host sums the
4 partials per batch, descales by 1/256, and adds bias + residual.

Matmuls run in fp8e4m3 DoubleRow perf mode (2 contraction subtiles per
instruction, 0.5 cycles/row): weights host-scaled by 16 (fp8 dynamic
range). The softmax exp is split across ACT and DVE per key-tile pair
(GPSIMD cannot access PSUM): ACT computes exp natively; DVE computes a
Schraudolph exp: uint8 = round(score*8*log2e*scale) bitcast to fp8e4m3
(float->uint8 saturates at 0 on HW, clamping the low tail). A constant
contraction row (qT/kT partition 40, value 48 -> +2304 in every psum
score) centers both paths on p~ = exp(s - 3.85), keeping the fp8 pt in
range for row-max scores up to ~9.3 (real data reaches ~8); the shared
bias cancels in the softmax normalization.

Phases: A projects q/k/v for both heads through a 4-slot PSUM ring; B is
one flat jg-stream over all 16 (head, chunk) softmaxes (PE emits score
pairs, ACT/DVE exponentiate, AV DoubleRow matmuls trail by AVLAG with the
ones-column at 96 giving the row sums); C runs the output projection.

Hardware rules learned the hard way (violations = NaN or dead device):
  - PE matmuls honor only ONE semaphore wait: tiny observer matmuls make
    PE see every DMA-queue semaphore once; engine assignment keeps each
    real matmul's remaining unobserved waits on a single semaphore.
  - GPSIMD cannot access PSUM (memsets only).
  - dual-fp8 Ldweights: subtile column count % 4 == 0 and subtile byte
    stride % 16 == 0 (hence VS=104), and 64 < rows < 128 is invalid
    (hence the non-DoubleRow output projection with 80 rows).
  - fp8 overflow (>240) produces inf/NaN, not saturation.
"""
import sys

if "/opt/trn_rl_repo" not in sys.path:
    sys.path.insert(0, "/opt/trn_rl_repo")

import numpy as np

B, S, C = 2, 4096, 640
H, D = 8, 80
HPC = 2          # heads per core
NCORES = 8
KC = 5           # real contraction tiles over C
KC6 = 6          # padded to even for DoubleRow pairs
NCB = 8          # hsT DMA column batches
WSCALE = 16.0    # host weight scale (fp8 dynamic range)
SCALE = 1.0 / float(np.sqrt(D))
SEFF = SCALE / (WSCALE * WSCALE)        # psum score -> true scaled score
SCHA = SEFF * 8.0 / float(np.log(2.0))  # Schraudolph slope (fp8e4m3, m=3)
# Schraudolph bias is embedded in the scores via a constant contraction row
# (qT/kT row 40, half 0, value 48.0 each -> +2304 in every psum score), so
# the uint8 cast input is >= 0 (no negative wrap; low tail clamps via max).
BROW = 48.0
BPSUM = BROW * BROW                     # 9216
# ACT path must encode the same value: exp(seff*psum + EBIAS) == 2^((i-56)/8)
EBIAS = float(-BPSUM * SEFF - (56.0 - BPSUM * SCHA) * np.log(2.0) / 8.0)
VS = 104  # dual-fp8 ldweights: cols % 4 == 0, subtile stride % 16 == 0
ONESCOL = 96

_NC_CACHE = {}


def build_nc(s=S):
    import concourse.bacc as bacc
    import concourse.mybir as mybir
    import concourse.tile as tile
    from concourse.tile import add_dep_helper

    f32 = mybir.dt.float32
    bf16 = mybir.dt.bfloat16
    fp8 = mybir.dt.float8e4
    u8 = mybir.dt.uint8
    DR = mybir.MatmulPerfMode.DoubleRow
    Exp = mybir.ActivationFunctionType.Exp

    njt = s // 128    # key tiles
    nit = s // 128    # output i-tiles
    nch = s // 512    # query chunks
    njg = njt // 2    # key-tile pairs per chunk
    cbw = s // NCB    # hsT column batch width
    assert s % 512 == 0 and njt % 4 == 0

    nc = bacc.Bacc("TRN2", target_bir_lowering=False, debug=False,
                   num_devices=NCORES)

    hsT = nc.dram_tensor("hsT", [128, NCB * KC6 * cbw], fp8,
                         kind="ExternalInput")
    wq = nc.dram_tensor("wq", [128, KC6 * HPC * D], fp8, kind="ExternalInput")
    wk = nc.dram_tensor("wk", [128, KC6 * HPC * D], fp8, kind="ExternalInput")
    wv = nc.dram_tensor("wv", [128, KC6 * HPC * D], fp8, kind="ExternalInput")
    wo = nc.dram_tensor("wo", [128, HPC * C], fp8, kind="ExternalInput")
    qkb = nc.dram_tensor("qkb", [1, 2 * s], fp8, kind="ExternalInput")
    o_dram = nc.dram_tensor("o", [128, nit * C], fp8,
                             kind="ExternalOutput")

    # engine-assignment helpers --------------------------------------------
    # exp halves per chunk: proportional-rate greedy schedule so each
    # engine's exp time per chunk is equal (ACT 612ns, DVE 658, Pool 806)
    # GPSIMD cannot access PSUM -> only ACT and DVE can read scores.
    _counts = {"A": 39, "D": 25}
    _cost = {"A": 1038.0, "D": 1192.0}
    _n64 = sum(_counts.values())
    EXP_PAT16 = []
    _load = {k: 0.0 for k in _counts}
    for _i in range(_n64):
        pick = max(_counts,
                   key=lambda k: (_i + 1) * _counts[k] / _n64
                   - _load[k] / _cost[k])
        _load[pick] += _cost[pick]
        EXP_PAT16.append(pick)
    for _i in range(0, _n64, 16):
        if EXP_PAT16[_i] != "D":
            for _j in range(_i + 1, _i + 16):
                if EXP_PAT16[_j] == "D":
                    EXP_PAT16[_j] = EXP_PAT16[_i]
                    EXP_PAT16[_i] = "D"
                    break
    cp_state = {"i": 0}

    with tile.TileContext(nc) as tc:
        with (
            tc.tile_pool(name="persist", bufs=1) as pp,
            tc.tile_pool(name="ppt", bufs=8) as ppt,
            tc.tile_pool(name="pbcs", bufs=2) as pbcs,
            tc.tile_pool(name="pobuf", bufs=2) as pobuf,
            tc.tile_pool(name="psc_ps", bufs=3, space="PSUM") as psc,
            tc.tile_pool(name="pprj_ps", bufs=1, space="PSUM") as pprj,
            tc.tile_pool(name="pav_ps", bufs=1, space="PSUM") as pav,
        ):
            # ---- persistent tiles ----
            hsT_sb = pp.tile([128, KC6, s], fp8, name="hsT_sb")
            wq_sb = pp.tile([128, KC6, HPC, 2, 40], fp8, name="wq_sb")
            wk_sb = pp.tile([128, KC6, HPC, 2, 40], fp8, name="wk_sb")
            wv_sb = pp.tile([128, KC6, HPC * D], fp8, name="wv_sb")
            wo_sb = pp.tile([128, HPC, C], fp8, name="wo_sb")
            qT = [pp.tile([128, 2, s], fp8, name=f"qT{h}") for h in range(HPC)]
            kT = [pp.tile([128, 2, s], fp8, name=f"kT{h}") for h in range(HPC)]
            v_sb = pp.tile([128, njt, HPC, VS], fp8, name="v_sb")
            avn = pp.tile([128, HPC, s], fp8, name="avn")
            recip_sb = pp.tile([128, 512], bf16, name="recip_sb")
            ones_sb = pp.tile([128, D], bf16, name="ones_sb")
            ebias = pp.tile([128, 1], f32, name="ebias")

            nc.vector.memset(ebias[:], EBIAS)
            nc.gpsimd.memset(recip_sb[:, :], 0.0)
            nc.gpsimd.memset(ones_sb[:, :], 0.0)
            nc.gpsimd.memset(ones_sb[0:1, :], 1.0)
            # v data cols 0:80 come from the projection; only the pad and the
            # denominator ones-column need initialization
            nc.gpsimd.memset(v_sb[:, :, :, D:VS], 0.0)
            nc.gpsimd.memset(v_sb[:, :, :, ONESCOL], 1.0)
            # ---- input DMAs (cb0 first so projections start early) ----
            def dma_cb(cb):
                nc.sync.dma_start(
                    hsT_sb[:, :, cb * cbw:(cb + 1) * cbw],
                    hsT[:, cb * KC6 * cbw:(cb + 1) * KC6 * cbw])

            dma_cb(0)
            nc.sync.dma_start(wk_sb.rearrange("p a b c d -> p (a b c d)"),
                              wk[:, :])
            nc.sync.dma_start(wv_sb.rearrange("p a b -> p (a b)"), wv[:, :])
            nc.sync.dma_start(wq_sb.rearrange("p a b c d -> p (a b c d)"),
                              wq[:, :])
            dma_cb(1)
            dma_cb(2)
            # softmax bias row at partition 40 (via DMA: engines can't
            # start an AP at a non-32-aligned partition)
            qkb_dmas = [nc.sync.dma_start(t[40:41, 0:2, :], qkb[:, :])
                        for t in qT + kT]
            for cb in range(3, NCB):
                dma_cb(cb)
            nc.sync.dma_start(wo_sb.rearrange("p a b -> p (a b)"), wo[:, :])

            def copy_eng(which=None):
                """Rotate copies across engines for balance."""
                if which == "A":
                    return nc.scalar
                if which == "D":
                    return nc.vector
                if which == "P":
                    return nc.gpsimd
                i = cp_state["i"] = cp_state["i"] + 1
                return (nc.scalar, nc.vector)[i % 2]

            def ecopy(eng, dst, src):
                if eng is nc.scalar:
                    eng.copy(dst, src)
                else:
                    eng.tensor_copy(dst, src)

            # ---- projection units (all-fp8 DoubleRow) ----
            prj_state = {"i": 0}

            def pe_observe(src_ap, extra_dep=None):
                """Tiny matmul so PE observes the semaphore guarding
                src_ap (PE matmuls only honor a single sync wait)."""
                dum = pprj.tile([8, 8], f32, name="dum", tag="prj")
                mm = nc.tensor.matmul(dum[:], src_ap, src_ap, start=True,
                                      stop=True, skip_group_check=True)
                if extra_dep is not None:
                    add_dep_helper(mm.ins, extra_dep.ins,
                                   reason="observe DMA sem on PE")

            def prj_tile(shape):
                i = prj_state["i"] = prj_state["i"] + 1
                if i % 4 == 3:
                    return pprj.tile(shape, f32, name="prj_ps", tag="prj")
                return psc.tile(shape, f32, name="prj_ps", tag="scslot")

            def emit_qk_chunk(h, w_sb, dst, iq, ceng=None):
                """dst[0:40, 0:2, iq*512:(iq+1)*512] = head-h projection."""
                i0 = iq * 512
                for half in range(2):
                    ps = prj_tile([40, 512])
                    for p in range(KC6 // 2):
                        nc.tensor.matmul(
                            ps[:],
                            w_sb[:, 2 * p:2 * p + 2, h, half, :],
                            hsT_sb[:, 2 * p:2 * p + 2, i0:i0 + 512],
                            start=(p == 0), stop=(p == KC6 // 2 - 1),
                            perf_mode=DR,
                        )
                    ecopy(copy_eng(ceng),
                          dst[0:40, half, i0:i0 + 512], ps[:])

            def emit_v_tile(jt, ceng=None):
                ps = prj_tile([128, HPC, D])
                for p in range(KC6 // 2):
                    nc.tensor.matmul(
                        ps.rearrange("p a b -> p (a b)"),
                        hsT_sb[:, 2 * p:2 * p + 2, jt * 128:(jt + 1) * 128],
                        wv_sb[:, 2 * p:2 * p + 2, :],
                        start=(p == 0), stop=(p == KC6 // 2 - 1),
                        perf_mode=DR,
                    )
                ecopy(copy_eng(ceng), v_sb[:, jt, 0:2, 0:D], ps[:, :, :])

            # ---- attention: flat jg stream across all chunks ----
            # (no per-chunk pipeline drain: exp engines stay fed across
            # chunk boundaries; AV matmuls trail by AVLAG positions)
            AVLAG = 4
            av_state = {}

            def emit_norm(h, i8):
                """Normalize chunk (h, i8): avn = av[0:D] / av[96]."""
                i0 = i8 * 512
                av = av_state.pop((h, i8))
                with nc.allow_low_precision(
                        reason="bf16 recip feeds broadcast matmul"):
                    nc.vector.reciprocal(recip_sb[0:1, 0:512],
                                         av[ONESCOL:ONESCOL + 1, :])
                av2 = pbcs.tile([D, 512], f32, name="av2")
                nc.vector.tensor_copy(av2[:], av[0:D, :])
                bc = pprj.tile([D, 512], f32, name="bc_ps", tag="prj")
                nc.tensor.matmul(bc[:], ones_sb[:], recip_sb[:, 0:512],
                                 start=True, stop=True)
                nc.vector.tensor_mul(avn[0:D, h, i0:i0 + 512], av2[:],
                                      bc[:])

            def attention_stream(chunks, filler_hook=None):
                pend = []
                n = len(chunks)
                for g in range(n * njg + AVLAG):
                    if g < n * njg:
                        h, i8 = chunks[g // njg]
                        jg = g % njg
                        i0 = i8 * 512
                        if filler_hook is not None and jg % 4 == 3:
                            filler_hook()
                        pt = ppt.tile([128, 2, 512], fp8, name="pt")
                        sc = psc.tile([128, 2, 512], f32, name="sc_ps",
                                      tag="scslot")
                        for jj in range(2):
                            j = 2 * jg + jj
                            nc.tensor.matmul(
                                sc[:, jj, :],
                                kT[h][0:41, 0:2, j * 128:(j + 1) * 128],
                                qT[h][0:41, 0:2, i0:i0 + 512],
                                start=True, stop=True,
                                perf_mode=DR,
                            )
                        if EXP_PAT16[g % 64] == "A":
                            nc.scalar.activation(
                                out=pt.rearrange("p a b -> p (a b)"),
                                in_=sc.rearrange("p a b -> p (a b)"),
                                func=Exp, scale=SEFF, bias=ebias[:],
                            )
                        else:
                            nc.vector.tensor_scalar(
                                pt.rearrange("p a b -> p (a b)").bitcast(u8),
                                sc.rearrange("p a b -> p (a b)"),
                                SCHA, 0.0,
                                op0=mybir.AluOpType.mult,
                                op1=mybir.AluOpType.max,
                            )
                        pend.append((h, i8, jg, pt))
                    if g >= AVLAG:
                        h2, i82, jg2, pt2 = pend.pop(0)
                        if jg2 == 0:
                            av_state[(h2, i82)] = pav.tile(
                                [VS, 512], f32, name="av_ps")
                        nc.tensor.matmul(
                            av_state[(h2, i82)][:],
                            v_sb[:, 2 * jg2:2 * jg2 + 2, h2, 0:VS],
                            pt2[:, :, :],
                            start=(jg2 == 0), stop=(jg2 == njg - 1),
                            perf_mode=DR,
                        )
                        if jg2 == njg - 1:
                            emit_norm(h2, i82)

            # ---- output projection ----
            o_state = {"buf": None}

            def emit_c_tile(g, ceng=None):
                if g % 4 == 0:
                    o_state["buf"] = pobuf.tile([128, 4, C], fp8,
                                                name="o_buf")
                o_buf = o_state["buf"]
                t0 = g * 128
                o_ps = psc.tile([128, C], f32, name="o_ps", tag="scslot")
                for n0, n1 in ((0, 512), (512, C)):
                    for hh in range(HPC):
                        nc.tensor.matmul(
                            o_ps[:, n0:n1], avn[0:D, hh, t0:t0 + 128],
                            wo_sb[0:D, hh, n0:n1],
                            start=(hh == 0), stop=(hh == HPC - 1),
                        )
                ecopy(copy_eng(ceng), o_buf[:, g % 4, :], o_ps[:])
                if g % 4 == 3:
                    nc.sync.dma_start(
                        o_dram[:, (g - 3) * C:(g + 1) * C],
                        o_buf.rearrange("p a b -> p (a b)"))

            # ============ Phase A: all projections ========================
            # PE must observe every input-DMA semaphore once (single-wait
            # rule) before real matmuls depend on them
            seen_cb = set()

            def observe_cb(cb):
                if cb not in seen_cb:
                    seen_cb.add(cb)
                    pe_observe(hsT_sb[0:8, 0, cb * cbw:cb * cbw + 8])

            pe_observe(wk_sb[0:8, 0, 0, 0, 0:8])
            pe_observe(wv_sb[0:8, 0, 0:8])
            pe_observe(wq_sb[0:8, 0, 0, 0, 0:8])
            observe_cb(0)

            def emit_qk_all(h, w_sb, dst, iq):
                for cb in range((iq * 512) // cbw,
                                ((iq + 1) * 512 - 1) // cbw + 1):
                    observe_cb(cb)
                emit_qk_chunk(h, w_sb, dst, iq)

            for iq in range(nch):
                emit_qk_all(0, wk_sb, kT[0], iq)
            for jt in range(njt):
                observe_cb((jt * 128) // cbw)
                emit_v_tile(jt)
            for iq in range(nch):
                emit_qk_all(0, wq_sb, qT[0], iq)
            for iq in range(nch):
                emit_qk_all(1, wk_sb, kT[1], iq)
            for iq in range(nch):
                emit_qk_all(1, wq_sb, qT[1], iq)
            # observe qkb bias rows and the tail projection copies on both
            # engines before attention consumes them
            for dma in qkb_dmas:
                pe_observe(ones_sb[0:8, 0:8], extra_dep=dma)
            pe_observe(qT[1][0:8, 1, s - 8:s])
            pe_observe(qT[1][0:8, 0, s - 8:s])
            pe_observe(kT[1][0:8, 1, s - 8:s])
            pe_observe(kT[1][0:8, 0, s - 8:s])
            pe_observe(qT[0][0:8, 1, s - 8:s])
            pe_observe(kT[0][0:8, 1, s - 8:s])
            pe_observe(v_sb[0:8, njt - 1, 1, 0:8])

            # ============ Phase B: attention (both heads) ==================
            attention_stream([(h, i8) for h in range(HPC)
                              for i8 in range(nch)])

            # ============ Phase C: out-projection ==========================
            pe_observe(wo_sb[0:8, 0, 0:8])
            pe_observe(avn[0:8, 1, s - 8:s])
            for g in range(nit):
                emit_c_tile(g)

    nc.compile()
    return nc


def _get_nc(s=S):
    if s not in _NC_CACHE:
        _NC_CACHE[s] = build_nc(s)
    return _NC_CACHE[s]


def make_in_maps(hidden_states, Wq, Wk, Wv, Wo, s=S):
    """Shard full inputs into 8 per-core fp8 input dicts."""
    import ml_dtypes
    fp8 = ml_dtypes.float8_e4m3

    cbw = s // NCB
    hs = np.asarray(hidden_states, dtype=np.float32)
    Wq = np.asarray(Wq, dtype=np.float32)
    Wk = np.asarray(Wk, dtype=np.float32)
    Wv = np.asarray(Wv, dtype=np.float32)
    Wo = np.asarray(Wo, dtype=np.float32)

    # hsT[p, cb, kc, u] = hs[b][cb*cbw+u, kc*128+p]; kc=5 zero
    hsTs = []
    for b in range(B):
        t = hs[b].T.reshape(KC, 128, NCB, cbw)  # [kc, p, cb, u]
        hp8 = np.zeros((128, NCB, KC6, cbw), np.float32)
        hp8[:, :, :KC, :] = t.transpose(1, 2, 0, 3)
        hsTs.append(hp8.reshape(128, NCB * KC6 * cbw).astype(fp8))

    def pack_qk(W, hp):
        # -> [128, KC6, HPC, 2, 40]
        out = np.zeros((128, KC6, HPC, 2, 40), np.float32)
        rows = W[HPC * D * hp:HPC * D * (hp + 1), :] * WSCALE  # [160, C]
        r = rows.reshape(HPC, 2, 40, KC, 128)
        out[:, :KC] = r.transpose(4, 3, 0, 1, 2)
        return np.ascontiguousarray(
            out.reshape(128, KC6 * HPC * D)).astype(fp8)

    def pack_v(W, hp):
        out = np.zeros((128, KC6, HPC * D), np.float32)
        rows = W[HPC * D * hp:HPC * D * (hp + 1), :] * WSCALE  # [160, C]
        r = rows.reshape(HPC * D, KC, 128)
        out[:, :KC] = r.transpose(2, 1, 0)
        return np.ascontiguousarray(out.reshape(128, KC6 * HPC * D)).astype(fp8)

    def pack_wo(W, hp):
        # wo[p(d), h, c] = 16*Wo[c, hp*160 + h*80 + p]
        out = np.zeros((128, HPC, C), np.float32)
        cols = W[:, HPC * D * hp:HPC * D * (hp + 1)] * WSCALE  # [C, 160]
        out[0:D] = cols.T.reshape(HPC, D, C).transpose(1, 0, 2)
        return np.ascontiguousarray(out.reshape(128, HPC * C)).astype(fp8)

    in_maps = []
    for c in range(NCORES):
        b, hp = divmod(c, NCORES // B)
        qkb = np.zeros((1, 2 * s), np.float32)
        qkb[0, :s] = BROW
        in_maps.append({
            "hsT": hsTs[b],
            "qkb": qkb.astype(fp8),
            "wq": pack_qk(Wq, hp),
            "wk": pack_qk(Wk, hp),
            "wv": pack_v(Wv, hp),
            "wo": pack_wo(Wo, hp),
        })
    return in_maps


def unpermute_o(o_core, s=S):
    """[128, (s/128)*C] partition-major bf16 -> [s, C] f32."""
    nit = s // 128
    return np.asarray(o_core, dtype=np.float32).reshape(
        128, nit, C).transpose(1, 0, 2).reshape(s, C)


def assemble(results, hidden_states, bo, s=S):
    hs = np.asarray(hidden_states, dtype=np.float32)
    bo = np.asarray(bo, dtype=np.float32)
    out = np.empty((B, s, C), dtype=np.float32)
    ncb = NCORES // B
    descale = 1.0 / (WSCALE * WSCALE)
    for b in range(B):
        acc = unpermute_o(results[b * ncb]["o"], s).astype(np.float64)
        for k in range(1, ncb):
            acc = acc + unpermute_o(results[b * ncb + k]["o"], s)
        out[b] = (acc * descale + bo[None, :]).astype(np.float32) + hs[b]
    return out


def kernel(hidden_states, Wq, Wk, Wv, Wo, bo):
    from concourse.bass_utils import run_bass_kernel_spmd

    nc = _get_nc(S)
    in_maps = make_in_maps(hidden_states, Wq, Wk, Wv, Wo)
    res = run_bass_kernel_spmd(nc, in_maps, core_ids=list(range(NCORES)))
    return assemble(res.results, hidden_states, bo)


# revision 9
# speedup vs baseline: 1.0222x; 1.0065x over previous
"""CombinedAttentionProcessor kernel for 8 Trainium2 NeuronCores (fp8).

Problem: B=2, S=4096, C=640, H=8 heads, D=80 head_dim.
    q/k/v = hs @ W{q,k,v}.T ; per-(b,h): softmax(q k^T / sqrt(D)) v ;
    out = attn @ Wo.T + bo + residual.

Sharding: 16 (batch, head) groups -> 2 per core. Each core computes its 2
heads' attention and a partial output projection [S, C]; the host sums the
4 partials per batch, descales by 1/256, and adds bias + residual.

Matmuls run in fp8e4m3 DoubleRow perf mode (2 contraction subtiles per
instruction, 0.5 cycles/row): weights host-scaled by 16 (fp8 dynamic
range). The softmax exp is split across ACT and DVE per key-tile pair
(GPSIMD cannot access PSUM): ACT computes exp natively; DVE computes a
Schraudolph exp: uint8 = round(score*8*log2e*scale) bitcast to fp8e4m3
(float->uint8 saturates at 0 on HW, clamping the low tail). A constant
contraction row (qT/kT partition 40, value 48 -> +2304 in every psum
score) centers both paths on p~ = exp(s - 3.85), keeping the fp8 pt in
range for row-max scores up to ~9.3 (real data reaches ~8); the shared
bias cancels in the softmax normalization.

Phases: A projects q/k/v for both heads through a 4-slot PSUM ring; B is
one flat jg-stream over all 16 (head, chunk) softmaxes (PE emits score
pairs, ACT/DVE exponentiate, AV DoubleRow matmuls trail by AVLAG with the
ones-column at 96 giving the row sums); C runs the output projection.

Hardware rules learned the hard way (violations = NaN or dead device):
  - PE matmuls honor only ONE semaphore wait: tiny observer matmuls make
    PE see every DMA-queue semaphore once; engine assignment keeps each
    real matmul's remaining unobserved waits on a single semaphore.
  - GPSIMD cannot access PSUM (memsets only).
  - dual-fp8 Ldweights: subtile column count % 4 == 0 and subtile byte
    stride % 16 == 0 (hence VS=104), and 64 < rows < 128 is invalid
    (hence the non-DoubleRow output projection with 80 rows).
  - fp8 overflow (>240) produces inf/NaN, not saturation.
"""
import sys

if "/opt/trn_rl_repo" not in sys.path:
    sys.path.insert(0, "/opt/trn_rl_repo")

import numpy as np

B, S, C = 2, 4096, 640
H, D = 8, 80
HPC = 2          # heads per core
NCORES = 8
KC = 5           # real contraction tiles over C
KC6 = 6          # padded to even for DoubleRow pairs
NCB = 8          # hsT DMA column batches
WSCALE = 16.0    # host weight scale (fp8 dynamic range)
SCALE = 1.0 / float(np.sqrt(D))
SEFF = SCALE / (WSCALE * WSCALE)        # psum score -> true scaled score
SCHA = SEFF * 8.0 / float(np.log(2.0))  # Schraudolph slope (fp8e4m3, m=3)
# Schraudolph bias is embedded in the scores via a constant contraction row
# (qT/kT row 40, half 0, value 48.0 each -> +2304 in every psum score), so
# the uint8 cast input is >= 0 (no negative wrap; low tail clamps via max).
BROW = 48.0
BPSUM = BROW * BROW                     # 2304
# ACT path must encode the same value: exp(seff*psum + EBIAS) == 2^((i-56)/8)
EBIAS = float(-BPSUM * SEFF - (56.0 - BPSUM * SCHA) * np.log(2.0) / 8.0)
VS = 104  # dual-fp8 ldweights: cols % 4 == 0, subtile stride % 16 == 0
ONESCOL = 96

_NC_CACHE = {}


def build_nc(s=S):
    import concourse.bacc as bacc
    import concourse.mybir as mybir
    import concourse.tile as tile
    from concourse.tile import add_dep_helper

    f32 = mybir.dt.float32
    bf16 = mybir.dt.bfloat16
    fp8 = mybir.dt.float8e4
    u8 = mybir.dt.uint8
    DR = mybir.MatmulPerfMode.DoubleRow
    Exp = mybir.ActivationFunctionType.Exp

    njt = s // 128    # key tiles
    nit = s // 128    # output i-tiles
    nch = s // 512    # query chunks
    njg = njt // 2    # key-tile pairs per chunk
    cbw = s // NCB    # hsT column batch width
    assert s % 512 == 0 and njt % 4 == 0

    nc = bacc.Bacc("TRN2", target_bir_lowering=False, debug=False,
                   num_devices=NCORES)

    hsT = nc.dram_tensor("hsT", [128, NCB * KC6 * cbw], fp8,
                         kind="ExternalInput")
    wq = nc.dram_tensor("wq", [128, KC6 * HPC * D], fp8, kind="ExternalInput")
    wk = nc.dram_tensor("wk", [128, KC6 * HPC * D], fp8, kind="ExternalInput")
    wv = nc.dram_tensor("wv", [128, KC6 * HPC * D], fp8, kind="ExternalInput")
    wo = nc.dram_tensor("wo", [128, HPC * C], fp8, kind="ExternalInput")
    qkb = nc.dram_tensor("qkb", [1, 2 * s], fp8, kind="ExternalInput")
    o_dram = nc.dram_tensor("o", [128, nit * C], fp8,
                             kind="ExternalOutput")

    # engine-assignment helpers --------------------------------------------
    # exp halves per chunk: proportional-rate greedy schedule so each
    # engine's exp time per chunk is equal (ACT 612ns, DVE 658, Pool 806)
    # GPSIMD cannot access PSUM -> only ACT and DVE can read scores.
    _counts = {"A": 38, "D": 26}
    _cost = {"A": 1038.0, "D": 1192.0}
    _n64 = sum(_counts.values())
    EXP_PAT16 = []
    _load = {k: 0.0 for k in _counts}
    for _i in range(_n64):
        pick = max(_counts,
                   key=lambda k: (_i + 1) * _counts[k] / _n64
                   - _load[k] / _cost[k])
        _load[pick] += _cost[pick]
        EXP_PAT16.append(pick)
    for _i in range(0, _n64, 16):
        if EXP_PAT16[_i] != "D":
            for _j in range(_i + 1, _i + 16):
                if EXP_PAT16[_j] == "D":
                    EXP_PAT16[_j] = EXP_PAT16[_i]
                    EXP_PAT16[_i] = "D"
                    break
    cp_state = {"i": 0}

    with tile.TileContext(nc) as tc:
        with (
            tc.tile_pool(name="persist", bufs=1) as pp,
            tc.tile_pool(name="ppt", bufs=8) as ppt,
            tc.tile_pool(name="pbcs", bufs=2) as pbcs,
            tc.tile_pool(name="pobuf", bufs=2) as pobuf,
            tc.tile_pool(name="psc_ps", bufs=3, space="PSUM") as psc,
            tc.tile_pool(name="pprj_ps", bufs=1, space="PSUM") as pprj,
            tc.tile_pool(name="pav_ps", bufs=1, space="PSUM") as pav,
        ):
            # ---- persistent tiles ----
            hsT_sb = pp.tile([128, KC6, s], fp8, name="hsT_sb")
            wq_sb = pp.tile([128, KC6, HPC, 2, 40], fp8, name="wq_sb")
            wk_sb = pp.tile([128, KC6, HPC, 2, 40], fp8, name="wk_sb")
            wv_sb = pp.tile([128, KC6, HPC * D], fp8, name="wv_sb")
            wo_sb = pp.tile([128, HPC, C], fp8, name="wo_sb")
            qT = [pp.tile([128, 2, s], fp8, name=f"qT{h}") for h in range(HPC)]
            kT = [pp.tile([128, 2, s], fp8, name=f"kT{h}") for h in range(HPC)]
            v_sb = pp.tile([128, njt, HPC, VS], fp8, name="v_sb")
            avn = pp.tile([128, HPC, s], fp8, name="avn")
            recip_sb = pp.tile([128, 512], bf16, name="recip_sb")
            ones_sb = pp.tile([128, D], bf16, name="ones_sb")
            ebias = pp.tile([128, 1], f32, name="ebias")

            nc.vector.memset(ebias[:], EBIAS)
            nc.gpsimd.memset(recip_sb[:, :], 0.0)
            nc.gpsimd.memset(ones_sb[:, :], 0.0)
            nc.gpsimd.memset(ones_sb[0:1, :], 1.0)
            # v data cols 0:80 come from the projection; only the pad and the
            # denominator ones-column need initialization
            nc.gpsimd.memset(v_sb[:, :, :, D:VS], 0.0)
            nc.gpsimd.memset(v_sb[:, :, :, ONESCOL], 1.0)
            # ---- input DMAs (cb0 first so projections start early) ----
            def dma_cb(cb):
                nc.sync.dma_start(
                    hsT_sb[:, :, cb * cbw:(cb + 1) * cbw],
                    hsT[:, cb * KC6 * cbw:(cb + 1) * KC6 * cbw])

            dma_cb(0)
            nc.sync.dma_start(wk_sb.rearrange("p a b c d -> p (a b c d)"),
                              wk[:, :])
            nc.sync.dma_start(wv_sb.rearrange("p a b -> p (a b)"), wv[:, :])
            nc.sync.dma_start(wq_sb.rearrange("p a b c d -> p (a b c d)"),
                              wq[:, :])
            dma_cb(1)
            dma_cb(2)
            # softmax bias row at partition 40 (via DMA: engines can't
            # start an AP at a non-32-aligned partition)
            qkb_dmas = [nc.sync.dma_start(t[40:41, 0:2, :], qkb[:, :])
                        for t in qT + kT]
            for cb in range(3, NCB):
                dma_cb(cb)
            nc.sync.dma_start(wo_sb.rearrange("p a b -> p (a b)"), wo[:, :])

            def copy_eng(which=None):
                """Rotate copies across engines for balance."""
                if which == "A":
                    return nc.scalar
                if which == "D":
                    return nc.vector
                if which == "P":
                    return nc.gpsimd
                i = cp_state["i"] = cp_state["i"] + 1
                return (nc.scalar, nc.vector)[i % 2]

            def ecopy(eng, dst, src):
                if eng is nc.scalar:
                    eng.copy(dst, src)
                else:
                    eng.tensor_copy(dst, src)

            # ---- projection units (all-fp8 DoubleRow) ----
            prj_state = {"i": 0}

            def pe_observe(src_ap, extra_dep=None):
                """Tiny matmul so PE observes the semaphore guarding
                src_ap (PE matmuls only honor a single sync wait)."""
                dum = pprj.tile([8, 8], f32, name="dum", tag="prj")
                mm = nc.tensor.matmul(dum[:], src_ap, src_ap, start=True,
                                      stop=True, skip_group_check=True)
                if extra_dep is not None:
                    add_dep_helper(mm.ins, extra_dep.ins,
                                   reason="observe DMA sem on PE")

            def prj_tile(shape):
                i = prj_state["i"] = prj_state["i"] + 1
                if i % 4 == 3:
                    return pprj.tile(shape, f32, name="prj_ps", tag="prj")
                return psc.tile(shape, f32, name="prj_ps", tag="scslot")

            def emit_qk_chunk(h, w_sb, dst, iq, ceng=None):
                """dst[0:40, 0:2, iq*512:(iq+1)*512] = head-h projection."""
                i0 = iq * 512
                for half in range(2):
                    ps = prj_tile([40, 512])
                    for p in range(KC6 // 2):
                        nc.tensor.matmul(
                            ps[:],
                            w_sb[:, 2 * p:2 * p + 2, h, half, :],
                            hsT_sb[:, 2 * p:2 * p + 2, i0:i0 + 512],
                            start=(p == 0), stop=(p == KC6 // 2 - 1),
                            perf_mode=DR,
                        )
                    ecopy(copy_eng(ceng),
                          dst[0:40, half, i0:i0 + 512], ps[:])

            def emit_v_tile(jt, ceng=None):
                ps = prj_tile([128, HPC, D])
                for p in range(KC6 // 2):
                    nc.tensor.matmul(
                        ps.rearrange("p a b -> p (a b)"),
                        hsT_sb[:, 2 * p:2 * p + 2, jt * 128:(jt + 1) * 128],
                        wv_sb[:, 2 * p:2 * p + 2, :],
                        start=(p == 0), stop=(p == KC6 // 2 - 1),
                        perf_mode=DR,
                    )
                ecopy(copy_eng(ceng), v_sb[:, jt, 0:2, 0:D], ps[:, :, :])

            # ---- attention: flat jg stream across all chunks ----
            # (no per-chunk pipeline drain: exp engines stay fed across
            # chunk boundaries; AV matmuls trail by AVLAG positions)
            AVLAG = 4
            av_state = {}

            def emit_norm(h, i8):
                """Normalize chunk (h, i8): avn = av[0:D] / av[96]."""
                i0 = i8 * 512
                av = av_state.pop((h, i8))
                with nc.allow_low_precision(
                        reason="bf16 recip feeds broadcast matmul"):
                    nc.vector.reciprocal(recip_sb[0:1, 0:512],
                                         av[ONESCOL:ONESCOL + 1, :])
                av2 = pbcs.tile([D, 512], f32, name="av2")
                nc.vector.tensor_copy(av2[:], av[0:D, :])
                bc = pprj.tile([D, 512], f32, name="bc_ps", tag="prj")
                nc.tensor.matmul(bc[:], ones_sb[:], recip_sb[:, 0:512],
                                 start=True, stop=True)
                nc.vector.tensor_mul(avn[0:D, h, i0:i0 + 512], av2[:],
                                      bc[:])

            def attention_stream(chunks, filler_hook=None):
                pend = []
                n = len(chunks)
                for g in range(n * njg + AVLAG):
                    if g < n * njg:
                        h, i8 = chunks[g // njg]
                        jg = g % njg
                        i0 = i8 * 512
                        if filler_hook is not None and jg % 4 == 3:
                            filler_hook()
                        pt = ppt.tile([128, 2, 512], fp8, name="pt")
                        sc = psc.tile([128, 2, 512], f32, name="sc_ps",
                                      tag="scslot")
                        for jj in range(2):
                            j = 2 * jg + jj
                            nc.tensor.matmul(
                                sc[:, jj, :],
                                kT[h][0:41, 0:2, j * 128:(j + 1) * 128],
                                qT[h][0:41, 0:2, i0:i0 + 512],
                                start=True, stop=True,
                                perf_mode=DR,
                            )
                        if EXP_PAT16[g % 64] == "A":
                            nc.scalar.activation(
                                out=pt.rearrange("p a b -> p (a b)"),
                                in_=sc.rearrange("p a b -> p (a b)"),
                                func=Exp, scale=SEFF, bias=ebias[:],
                            )
                        else:
                            nc.vector.tensor_scalar(
                                pt.rearrange("p a b -> p (a b)").bitcast(u8),
                                sc.rearrange("p a b -> p (a b)"),
                                SCHA, 0.0,
                                op0=mybir.AluOpType.mult,
                                op1=mybir.AluOpType.max,
                            )
                        pend.append((h, i8, jg, pt))
                    if g >= AVLAG:
                        h2, i82, jg2, pt2 = pend.pop(0)
                        if jg2 == 0:
                            av_state[(h2, i82)] = pav.tile(
                                [VS, 512], f32, name="av_ps")
                        nc.tensor.matmul(
                            av_state[(h2, i82)][:],
                            v_sb[:, 2 * jg2:2 * jg2 + 2, h2, 0:VS],
                            pt2[:, :, :],
                            start=(jg2 == 0), stop=(jg2 == njg - 1),
                            perf_mode=DR,
                        )
                        if jg2 == njg - 1:
                            emit_norm(h2, i82)

            # ---- output projection ----
            o_state = {"buf": None}

            def emit_c_tile(g, ceng=None):
                if g % 4 == 0:
                    o_state["buf"] = pobuf.tile([128, 4, C], fp8,
                                                name="o_buf")
                o_buf = o_state["buf"]
                t0 = g * 128
                o_ps = psc.tile([128, C], f32, name="o_ps", tag="scslot")
                for n0, n1 in ((0, 512), (512, C)):
                    for hh in range(HPC):
                        nc.tensor.matmul(
                            o_ps[:, n0:n1], avn[0:D, hh, t0:t0 + 128],
                            wo_sb[0:D, hh, n0:n1],
                            start=(hh == 0), stop=(hh == HPC - 1),
                        )
                ecopy(copy_eng(ceng), o_buf[:, g % 4, :], o_ps[:])
                if g % 4 == 3:
                    nc.sync.dma_start(
                        o_dram[:, (g - 3) * C:(g + 1) * C],
                        o_buf.rearrange("p a b -> p (a b)"))

            # ============ Phase A: all projections ========================
            # PE must observe every input-DMA semaphore once (single-wait
            # rule) before real matmuls depend on them
            seen_cb = set()

            def observe_cb(cb):
                if cb not in seen_cb:
                    seen_cb.add(cb)
                    pe_observe(hsT_sb[0:8, 0, cb * cbw:cb * cbw + 8])

            pe_observe(wk_sb[0:8, 0, 0, 0, 0:8])
            pe_observe(wv_sb[0:8, 0, 0:8])
            pe_observe(wq_sb[0:8, 0, 0, 0, 0:8])
            observe_cb(0)

            def emit_qk_all(h, w_sb, dst, iq):
                for cb in range((iq * 512) // cbw,
                                ((iq + 1) * 512 - 1) // cbw + 1):
                    observe_cb(cb)
                emit_qk_chunk(h, w_sb, dst, iq)

            for iq in range(nch):
                emit_qk_all(0, wk_sb, kT[0], iq)
            for jt in range(njt):
                observe_cb((jt * 128) // cbw)
                emit_v_tile(jt)
            for iq in range(nch):
                emit_qk_all(0, wq_sb, qT[0], iq)
            for iq in range(nch):
                emit_qk_all(1, wk_sb, kT[1], iq)
            for iq in range(nch):
                emit_qk_all(1, wq_sb, qT[1], iq)
            # observe qkb bias rows and the tail projection copies on both
            # engines before attention consumes them
            for dma in qkb_dmas:
                pe_observe(ones_sb[0:8, 0:8], extra_dep=dma)
            pe_observe(qT[1][0:8, 1, s - 8:s])
            pe_observe(qT[1][0:8, 0, s - 8:s])
            pe_observe(kT[1][0:8, 1, s - 8:s])
            pe_observe(kT[1][0:8, 0, s - 8:s])
            pe_observe(qT[0][0:8, 1, s - 8:s])
            pe_observe(kT[0][0:8, 1, s - 8:s])
            pe_observe(v_sb[0:8, njt - 1, 1, 0:8])

            # ============ Phase B: attention (both heads) ==================
            attention_stream([(h, i8) for h in range(HPC)
                              for i8 in range(nch)])

            # ============ Phase C: out-projection ==========================
            pe_observe(wo_sb[0:8, 0, 0:8])
            pe_observe(avn[0:8, 1, s - 8:s])
            for g in range(nit):
                emit_c_tile(g)

    nc.compile()
    return nc


def _get_nc(s=S):
    if s not in _NC_CACHE:
        _NC_CACHE[s] = build_nc(s)
    return _NC_CACHE[s]


def make_in_maps(hidden_states, Wq, Wk, Wv, Wo, s=S):
    """Shard full inputs into 8 per-core fp8 input dicts."""
    import ml_dtypes
    fp8 = ml_dtypes.float8_e4m3

    cbw = s // NCB
    hs = np.asarray(hidden_states, dtype=np.float32)
    Wq = np.asarray(Wq, dtype=np.float32)
    Wk = np.asarray(Wk, dtype=np.float32)
    Wv = np.asarray(Wv, dtype=np.float32)
    Wo = np.asarray(Wo, dtype=np.float32)

    # hsT[p, cb, kc, u] = hs[b][cb*cbw+u, kc*128+p]; kc=5 zero
    hsTs = []
    for b in range(B):
        t = hs[b].T.reshape(KC, 128, NCB, cbw)  # [kc, p, cb, u]
        hp8 = np.zeros((128, NCB, KC6, cbw), np.float32)
        hp8[:, :, :KC, :] = t.transpose(1, 2, 0, 3)
        hsTs.append(hp8.reshape(128, NCB * KC6 * cbw).astype(fp8))

    def pack_qk(W, hp):
        # -> [128, KC6, HPC, 2, 40]
        out = np.zeros((128, KC6, HPC, 2, 40), np.float32)
        rows = W[HPC * D * hp:HPC * D * (hp + 1), :] * WSCALE  # [160, C]
        r = rows.reshape(HPC, 2, 40, KC, 128)
        out[:, :KC] = r.transpose(4, 3, 0, 1, 2)
        return np.ascontiguousarray(
            out.reshape(128, KC6 * HPC * D)).astype(fp8)

    def pack_v(W, hp):
        out = np.zeros((128, KC6, HPC * D), np.float32)
        rows = W[HPC * D * hp:HPC * D * (hp + 1), :] * WSCALE  # [160, C]
        r = rows.reshape(HPC * D, KC, 128)
        out[:, :KC] = r.transpose(2, 1, 0)
        return np.ascontiguousarray(out.reshape(128, KC6 * HPC * D)).astype(fp8)

    def pack_wo(W, hp):
        # wo[p(d), h, c] = 16*Wo[c, hp*160 + h*80 + p]
        out = np.zeros((128, HPC, C), np.float32)
        cols = W[:, HPC * D * hp:HPC * D * (hp + 1)] * WSCALE  # [C, 160]
        out[0:D] = cols.T.reshape(HPC, D, C).transpose(1, 0, 2)
        return np.ascontiguousarray(out.reshape(128, HPC * C)).astype(fp8)

    in_maps = []
    for c in range(NCORES):
        b, hp = divmod(c, NCORES // B)
        qkb = np.zeros((1, 2 * s), np.float32)
        qkb[0, :s] = BROW
        in_maps.append({
            "hsT": hsTs[b],
            "qkb": qkb.astype(fp8),
            "wq": pack_qk(Wq, hp),
            "wk": pack_qk(Wk, hp),
            "wv": pack_v(Wv, hp),
            "wo": pack_wo(Wo, hp),
        })
    return in_maps


def unpermute_o(o_core, s=S):
    """[128, (s/128)*C] partition-major bf16 -> [s, C] f32."""
    nit = s // 128
    return np.asarray(o_core, dtype=np.float32).reshape(
        128, nit, C).transpose(1, 0, 2).reshape(s, C)


def assemble(results, hidden_states, bo, s=S):
    hs = np.asarray(hidden_states, dtype=np.float32)
    bo = np.asarray(bo, dtype=np.float32)
    out = np.empty((B, s, C), dtype=np.float32)
    ncb = NCORES // B
    descale = 1.0 / (WSCALE * WSCALE)
    for b in range(B):
        acc = unpermute_o(results[b * ncb]["o"], s).astype(np.float64)
        for k in range(1, ncb):
            acc = acc + unpermute_o(results[b * ncb + k]["o"], s)
        out[b] = (acc * descale + bo[None, :]).astype(np.float32) + hs[b]
    return out


def kernel(hidden_states, Wq, Wk, Wv, Wo, bo):
    from concourse.bass_utils import run_bass_kernel_spmd

    nc = _get_nc(S)
    in_maps = make_in_maps(hidden_states, Wq, Wk, Wv, Wo)
    res = run_bass_kernel_spmd(nc, in_maps, core_ids=list(range(NCORES)))
    return assemble(res.results, hidden_states, bo)


# revision 10
# speedup vs baseline: 1.0293x; 1.0070x over previous
"""CombinedAttentionProcessor kernel for 8 Trainium2 NeuronCores (fp8).

Problem: B=2, S=4096, C=640, H=8 heads, D=80 head_dim.
    q/k/v = hs @ W{q,k,v}.T ; per-(b,h): softmax(q k^T / sqrt(D)) v ;
    out = attn @ Wo.T + bo + residual.

Sharding: 16 (batch, head) groups -> 2 per core. Each core computes its 2
heads' attention and a partial output projection [S, C]; the host sums the
4 partials per batch, descales by 1/256, and adds bias + residual.

Matmuls run in fp8e4m3 DoubleRow perf mode (2 contraction subtiles per
instruction, 0.5 cycles/row): weights host-scaled by 16 (fp8 dynamic
range). The softmax exp is split across ACT and DVE per key-tile pair
(GPSIMD cannot access PSUM): ACT computes exp natively; DVE computes a
Schraudolph exp: uint8 = round(score*8*log2e*scale) bitcast to fp8e4m3
(float->uint8 saturates at 0 on HW, clamping the low tail). A constant
contraction row (qT/kT partition 40, value 48 -> +2304 in every psum
score) centers both paths on p~ = exp(s - 3.85), keeping the fp8 pt in
range for row-max scores up to ~9.3 (real data reaches ~8); the shared
bias cancels in the softmax normalization.

Phases: A projects q/k/v for both heads through a 4-slot PSUM ring; B is
one flat jg-stream over all 16 (head, chunk) softmaxes (PE emits score
pairs, ACT/DVE exponentiate, AV DoubleRow matmuls trail by AVLAG with the
ones-column at 96 giving the row sums); C runs the output projection.

Hardware rules learned the hard way (violations = NaN or dead device):
  - PE matmuls honor only ONE semaphore wait: tiny observer matmuls make
    PE see every DMA-queue semaphore once; engine assignment keeps each
    real matmul's remaining unobserved waits on a single semaphore.
  - GPSIMD cannot access PSUM (memsets only).
  - dual-fp8 Ldweights: subtile column count % 4 == 0 and subtile byte
    stride % 16 == 0 (hence VS=104), and 64 < rows < 128 is invalid
    (hence the non-DoubleRow output projection with 80 rows).
  - fp8 overflow (>240) produces inf/NaN, not saturation.
"""
import sys

if "/opt/trn_rl_repo" not in sys.path:
    sys.path.insert(0, "/opt/trn_rl_repo")

import numpy as np

B, S, C = 2, 4096, 640
H, D = 8, 80
HPC = 2          # heads per core
NCORES = 8
KC = 5           # real contraction tiles over C
KC6 = 6          # padded to even for DoubleRow pairs
NCB = 8          # hsT DMA column batches
WSCALE = 16.0    # host weight scale (fp8 dynamic range)
SCALE = 1.0 / float(np.sqrt(D))
SEFF = SCALE / (WSCALE * WSCALE)        # psum score -> true scaled score
SCHA = SEFF * 8.0 / float(np.log(2.0))  # Schraudolph slope (fp8e4m3, m=3)
# Schraudolph bias is embedded in the scores via a constant contraction row
# (qT/kT row 40, half 0, value 48.0 each -> +2304 in every psum score), so
# the uint8 cast input is >= 0 (no negative wrap; low tail clamps via max).
BROW = 48.0
BPSUM = BROW * BROW                     # 2304
# ACT path must encode the same value: exp(seff*psum + EBIAS) == 2^((i-56)/8)
EBIAS = float(-BPSUM * SEFF - (56.0 - BPSUM * SCHA) * np.log(2.0) / 8.0)
VS = 104  # dual-fp8 ldweights: cols % 4 == 0, subtile stride % 16 == 0
ONESCOL = 96

_NC_CACHE = {}


def build_nc(s=S):
    import concourse.bacc as bacc
    import concourse.mybir as mybir
    import concourse.tile as tile
    from concourse.tile import add_dep_helper

    f32 = mybir.dt.float32
    bf16 = mybir.dt.bfloat16
    fp8 = mybir.dt.float8e4
    u8 = mybir.dt.uint8
    DR = mybir.MatmulPerfMode.DoubleRow
    Exp = mybir.ActivationFunctionType.Exp

    njt = s // 128    # key tiles
    nit = s // 128    # output i-tiles
    nch = s // 512    # query chunks
    njg = njt // 2    # key-tile pairs per chunk
    cbw = s // NCB    # hsT column batch width
    assert s % 512 == 0 and njt % 4 == 0

    nc = bacc.Bacc("TRN2", target_bir_lowering=False, debug=False,
                   num_devices=NCORES)

    hsT = nc.dram_tensor("hsT", [128, NCB * KC6 * cbw], fp8,
                         kind="ExternalInput")
    wq = nc.dram_tensor("wq", [128, KC6 * HPC * D], fp8, kind="ExternalInput")
    wk = nc.dram_tensor("wk", [128, KC6 * HPC * D], fp8, kind="ExternalInput")
    wv = nc.dram_tensor("wv", [128, KC6 * HPC * D], fp8, kind="ExternalInput")
    wo = nc.dram_tensor("wo", [128, HPC * C], fp8, kind="ExternalInput")
    qkb = nc.dram_tensor("qkb", [1, 2 * s], fp8, kind="ExternalInput")
    o_dram = nc.dram_tensor("o", [128, nit * C], fp8,
                             kind="ExternalOutput")

    # engine-assignment helpers --------------------------------------------
    # exp halves per chunk: proportional-rate greedy schedule so each
    # engine's exp time per chunk is equal (ACT 612ns, DVE 658, Pool 806)
    # GPSIMD cannot access PSUM -> only ACT and DVE can read scores.
    _counts = {"A": 38, "D": 26}
    _cost = {"A": 1038.0, "D": 1192.0}
    _n64 = sum(_counts.values())
    EXP_PAT16 = []
    _load = {k: 0.0 for k in _counts}
    for _i in range(_n64):
        pick = max(_counts,
                   key=lambda k: (_i + 1) * _counts[k] / _n64
                   - _load[k] / _cost[k])
        _load[pick] += _cost[pick]
        EXP_PAT16.append(pick)
    for _i in range(0, _n64, 16):
        if EXP_PAT16[_i] != "D":
            for _j in range(_i + 1, _i + 16):
                if EXP_PAT16[_j] == "D":
                    EXP_PAT16[_j] = EXP_PAT16[_i]
                    EXP_PAT16[_i] = "D"
                    break
    cp_state = {"i": 0}

    with tile.TileContext(nc) as tc:
        with (
            tc.tile_pool(name="persist", bufs=1) as pp,
            tc.tile_pool(name="ppt", bufs=8) as ppt,
            tc.tile_pool(name="pbcs", bufs=2) as pbcs,
            tc.tile_pool(name="pobuf", bufs=2) as pobuf,
            tc.tile_pool(name="psc_ps", bufs=3, space="PSUM") as psc,
            tc.tile_pool(name="pprj_ps", bufs=1, space="PSUM") as pprj,
            tc.tile_pool(name="pav_ps", bufs=1, space="PSUM") as pav,
        ):
            # ---- persistent tiles ----
            hsT_sb = pp.tile([128, KC6, s], fp8, name="hsT_sb")
            wq_sb = pp.tile([128, KC6, HPC, 2, 40], fp8, name="wq_sb")
            wk_sb = pp.tile([128, KC6, HPC, 2, 40], fp8, name="wk_sb")
            wv_sb = pp.tile([128, KC6, HPC * D], fp8, name="wv_sb")
            wo_sb = pp.tile([128, HPC, C], fp8, name="wo_sb")
            qT = [pp.tile([128, 2, s], fp8, name=f"qT{h}") for h in range(HPC)]
            kT = [pp.tile([128, 2, s], fp8, name=f"kT{h}") for h in range(HPC)]
            v_sb = pp.tile([128, njt, HPC, VS], fp8, name="v_sb")
            avn = pp.tile([128, HPC, s], fp8, name="avn")
            recip_sb = pp.tile([128, 512], bf16, name="recip_sb")
            ones_sb = pp.tile([128, D], bf16, name="ones_sb")
            ebias = pp.tile([128, 1], f32, name="ebias")

            nc.vector.memset(ebias[:], EBIAS)
            nc.gpsimd.memset(recip_sb[:, :], 0.0)
            nc.gpsimd.memset(ones_sb[:, :], 0.0)
            nc.gpsimd.memset(ones_sb[0:1, :], 1.0)
            # v data cols 0:80 come from the projection; only the pad and the
            # denominator ones-column need initialization
            nc.gpsimd.memset(v_sb[:, :, :, D:VS], 0.0)
            nc.gpsimd.memset(v_sb[:, :, :, ONESCOL], 1.0)
            # zero avn pad rows so the out-projection can run DoubleRow
            # with a full 128-row contraction (64<rows<128 is illegal)
            nc.gpsimd.memset(avn[64:128, :, :], 0.0)
            # ---- input DMAs (cb0 first so projections start early) ----
            def dma_cb(cb):
                nc.sync.dma_start(
                    hsT_sb[:, :, cb * cbw:(cb + 1) * cbw],
                    hsT[:, cb * KC6 * cbw:(cb + 1) * KC6 * cbw])

            dma_cb(0)
            nc.sync.dma_start(wk_sb.rearrange("p a b c d -> p (a b c d)"),
                              wk[:, :])
            nc.sync.dma_start(wv_sb.rearrange("p a b -> p (a b)"), wv[:, :])
            nc.sync.dma_start(wq_sb.rearrange("p a b c d -> p (a b c d)"),
                              wq[:, :])
            dma_cb(1)
            dma_cb(2)
            # softmax bias row at partition 40 (via DMA: engines can't
            # start an AP at a non-32-aligned partition)
            qkb_dmas = [nc.sync.dma_start(t[40:41, 0:2, :], qkb[:, :])
                        for t in qT + kT]
            for cb in range(3, NCB):
                dma_cb(cb)
            nc.sync.dma_start(wo_sb.rearrange("p a b -> p (a b)"), wo[:, :])

            def copy_eng(which=None):
                """Rotate copies across engines for balance."""
                if which == "A":
                    return nc.scalar
                if which == "D":
                    return nc.vector
                if which == "P":
                    return nc.gpsimd
                i = cp_state["i"] = cp_state["i"] + 1
                return (nc.scalar, nc.vector)[i % 2]

            def ecopy(eng, dst, src):
                if eng is nc.scalar:
                    eng.copy(dst, src)
                else:
                    eng.tensor_copy(dst, src)

            # ---- projection units (all-fp8 DoubleRow) ----
            prj_state = {"i": 0}

            def pe_observe(src_ap, extra_dep=None):
                """Tiny matmul so PE observes the semaphore guarding
                src_ap (PE matmuls only honor a single sync wait)."""
                dum = pprj.tile([8, 8], f32, name="dum", tag="prj")
                mm = nc.tensor.matmul(dum[:], src_ap, src_ap, start=True,
                                      stop=True, skip_group_check=True)
                if extra_dep is not None:
                    add_dep_helper(mm.ins, extra_dep.ins,
                                   reason="observe DMA sem on PE")

            def prj_tile(shape):
                i = prj_state["i"] = prj_state["i"] + 1
                if i % 4 == 3:
                    return pprj.tile(shape, f32, name="prj_ps", tag="prj")
                return psc.tile(shape, f32, name="prj_ps", tag="scslot")

            def emit_qk_chunk(h, w_sb, dst, iq, ceng=None):
                """dst[0:40, 0:2, iq*512:(iq+1)*512] = head-h projection."""
                i0 = iq * 512
                for half in range(2):
                    ps = prj_tile([40, 512])
                    for p in range(KC6 // 2):
                        nc.tensor.matmul(
                            ps[:],
                            w_sb[:, 2 * p:2 * p + 2, h, half, :],
                            hsT_sb[:, 2 * p:2 * p + 2, i0:i0 + 512],
                            start=(p == 0), stop=(p == KC6 // 2 - 1),
                            perf_mode=DR,
                        )
                    ecopy(copy_eng(ceng),
                          dst[0:40, half, i0:i0 + 512], ps[:])

            def emit_v_tile(jt, ceng=None):
                ps = prj_tile([128, HPC, D])
                for p in range(KC6 // 2):
                    nc.tensor.matmul(
                        ps.rearrange("p a b -> p (a b)"),
                        hsT_sb[:, 2 * p:2 * p + 2, jt * 128:(jt + 1) * 128],
                        wv_sb[:, 2 * p:2 * p + 2, :],
                        start=(p == 0), stop=(p == KC6 // 2 - 1),
                        perf_mode=DR,
                    )
                ecopy(copy_eng(ceng), v_sb[:, jt, 0:2, 0:D], ps[:, :, :])

            # ---- attention: flat jg stream across all chunks ----
            # (no per-chunk pipeline drain: exp engines stay fed across
            # chunk boundaries; AV matmuls trail by AVLAG positions)
            AVLAG = 4
            av_state = {}

            def emit_norm(h, i8):
                """Normalize chunk (h, i8): avn = av[0:D] / av[96]."""
                i0 = i8 * 512
                av = av_state.pop((h, i8))
                with nc.allow_low_precision(
                        reason="bf16 recip feeds broadcast matmul"):
                    nc.vector.reciprocal(recip_sb[0:1, 0:512],
                                         av[ONESCOL:ONESCOL + 1, :])
                av2 = pbcs.tile([D, 512], f32, name="av2")
                nc.vector.tensor_copy(av2[:], av[0:D, :])
                bc = pprj.tile([D, 512], f32, name="bc_ps", tag="prj")
                nc.tensor.matmul(bc[:], ones_sb[:], recip_sb[:, 0:512],
                                 start=True, stop=True)
                nc.vector.tensor_mul(avn[0:D, h, i0:i0 + 512], av2[:],
                                      bc[:])

            def attention_stream(chunks, filler_hook=None):
                pend = []
                n = len(chunks)
                for g in range(n * njg + AVLAG):
                    if g < n * njg:
                        h, i8 = chunks[g // njg]
                        jg = g % njg
                        i0 = i8 * 512
                        if filler_hook is not None and jg % 4 == 3:
                            filler_hook()
                        pt = ppt.tile([128, 2, 512], fp8, name="pt")
                        sc = psc.tile([128, 2, 512], f32, name="sc_ps",
                                      tag="scslot")
                        for jj in range(2):
                            j = 2 * jg + jj
                            nc.tensor.matmul(
                                sc[:, jj, :],
                                kT[h][0:41, 0:2, j * 128:(j + 1) * 128],
                                qT[h][0:41, 0:2, i0:i0 + 512],
                                start=True, stop=True,
                                perf_mode=DR,
                            )
                        if EXP_PAT16[g % 64] == "A":
                            nc.scalar.activation(
                                out=pt.rearrange("p a b -> p (a b)"),
                                in_=sc.rearrange("p a b -> p (a b)"),
                                func=Exp, scale=SEFF, bias=ebias[:],
                            )
                        else:
                            nc.vector.tensor_scalar(
                                pt.rearrange("p a b -> p (a b)").bitcast(u8),
                                sc.rearrange("p a b -> p (a b)"),
                                SCHA, 0.0,
                                op0=mybir.AluOpType.mult,
                                op1=mybir.AluOpType.max,
                            )
                        pend.append((h, i8, jg, pt))
                    if g >= AVLAG:
                        h2, i82, jg2, pt2 = pend.pop(0)
                        if jg2 == 0:
                            av_state[(h2, i82)] = pav.tile(
                                [VS, 512], f32, name="av_ps")
                        nc.tensor.matmul(
                            av_state[(h2, i82)][:],
                            v_sb[:, 2 * jg2:2 * jg2 + 2, h2, 0:VS],
                            pt2[:, :, :],
                            start=(jg2 == 0), stop=(jg2 == njg - 1),
                            perf_mode=DR,
                        )
                        if jg2 == njg - 1:
                            emit_norm(h2, i82)

            # ---- output projection ----
            o_state = {"buf": None}

            def emit_c_tile(g, ceng=None):
                if g % 4 == 0:
                    o_state["buf"] = pobuf.tile([128, 4, C], fp8,
                                                name="o_buf")
                o_buf = o_state["buf"]
                t0 = g * 128
                o_ps = psc.tile([128, C], f32, name="o_ps", tag="scslot")
                for n0, n1 in ((0, 512), (512, C)):
                    nc.tensor.matmul(
                        o_ps[:, n0:n1], avn[0:128, 0:2, t0:t0 + 128],
                        wo_sb[0:128, :, n0:n1],
                        start=True, stop=True, perf_mode=DR,
                    )
                ecopy(copy_eng(ceng), o_buf[:, g % 4, :], o_ps[:])
                if g % 4 == 3:
                    nc.sync.dma_start(
                        o_dram[:, (g - 3) * C:(g + 1) * C],
                        o_buf.rearrange("p a b -> p (a b)"))

            # ============ Phase A: all projections ========================
            # PE must observe every input-DMA semaphore once (single-wait
            # rule) before real matmuls depend on them
            seen_cb = set()

            def observe_cb(cb):
                if cb not in seen_cb:
                    seen_cb.add(cb)
                    pe_observe(hsT_sb[0:8, 0, cb * cbw:cb * cbw + 8])

            pe_observe(wk_sb[0:8, 0, 0, 0, 0:8])
            pe_observe(wv_sb[0:8, 0, 0:8])
            pe_observe(wq_sb[0:8, 0, 0, 0, 0:8])
            observe_cb(0)

            def emit_qk_all(h, w_sb, dst, iq):
                for cb in range((iq * 512) // cbw,
                                ((iq + 1) * 512 - 1) // cbw + 1):
                    observe_cb(cb)
                emit_qk_chunk(h, w_sb, dst, iq)

            for iq in range(nch):
                emit_qk_all(0, wk_sb, kT[0], iq)
            for jt in range(njt):
                observe_cb((jt * 128) // cbw)
                emit_v_tile(jt)
            for iq in range(nch):
                emit_qk_all(0, wq_sb, qT[0], iq)
            for iq in range(nch):
                emit_qk_all(1, wk_sb, kT[1], iq)
            for iq in range(nch):
                emit_qk_all(1, wq_sb, qT[1], iq)
            # observe qkb bias rows and the tail projection copies on both
            # engines before attention consumes them
            for dma in qkb_dmas:
                pe_observe(ones_sb[0:8, 0:8], extra_dep=dma)
            pe_observe(qT[1][0:8, 1, s - 8:s])
            pe_observe(qT[1][0:8, 0, s - 8:s])
            pe_observe(kT[1][0:8, 1, s - 8:s])
            pe_observe(kT[1][0:8, 0, s - 8:s])
            pe_observe(qT[0][0:8, 1, s - 8:s])
            pe_observe(kT[0][0:8, 1, s - 8:s])
            pe_observe(v_sb[0:8, njt - 1, 1, 0:8])

            # ============ Phase B: attention (both heads) ==================
            attention_stream([(h, i8) for h in range(HPC)
                              for i8 in range(nch)])

            # ============ Phase C: out-projection ==========================
            pe_observe(wo_sb[0:8, 0, 0:8])
            pe_observe(avn[0:8, 1, s - 8:s])
            for g in range(nit):
                emit_c_tile(g)

    nc.compile()
    return nc


def _get_nc(s=S):
    if s not in _NC_CACHE:
        _NC_CACHE[s] = build_nc(s)
    return _NC_CACHE[s]


def make_in_maps(hidden_states, Wq, Wk, Wv, Wo, s=S):
    """Shard full inputs into 8 per-core fp8 input dicts."""
    import ml_dtypes
    fp8 = ml_dtypes.float8_e4m3

    cbw = s // NCB
    hs = np.asarray(hidden_states, dtype=np.float32)
    Wq = np.asarray(Wq, dtype=np.float32)
    Wk = np.asarray(Wk, dtype=np.float32)
    Wv = np.asarray(Wv, dtype=np.float32)
    Wo = np.asarray(Wo, dtype=np.float32)

    # hsT[p, cb, kc, u] = hs[b][cb*cbw+u, kc*128+p]; kc=5 zero
    hsTs = []
    for b in range(B):
        t = hs[b].T.reshape(KC, 128, NCB, cbw)  # [kc, p, cb, u]
        hp8 = np.zeros((128, NCB, KC6, cbw), np.float32)
        hp8[:, :, :KC, :] = t.transpose(1, 2, 0, 3)
        hsTs.append(hp8.reshape(128, NCB * KC6 * cbw).astype(fp8))

    def pack_qk(W, hp):
        # -> [128, KC6, HPC, 2, 40]
        out = np.zeros((128, KC6, HPC, 2, 40), np.float32)
        rows = W[HPC * D * hp:HPC * D * (hp + 1), :] * WSCALE  # [160, C]
        r = rows.reshape(HPC, 2, 40, KC, 128)
        out[:, :KC] = r.transpose(4, 3, 0, 1, 2)
        return np.ascontiguousarray(
            out.reshape(128, KC6 * HPC * D)).astype(fp8)

    def pack_v(W, hp):
        out = np.zeros((128, KC6, HPC * D), np.float32)
        rows = W[HPC * D * hp:HPC * D * (hp + 1), :] * WSCALE  # [160, C]
        r = rows.reshape(HPC * D, KC, 128)
        out[:, :KC] = r.transpose(2, 1, 0)
        return np.ascontiguousarray(out.reshape(128, KC6 * HPC * D)).astype(fp8)

    def pack_wo(W, hp):
        # wo[p(d), h, c] = 16*Wo[c, hp*160 + h*80 + p]
        out = np.zeros((128, HPC, C), np.float32)
        cols = W[:, HPC * D * hp:HPC * D * (hp + 1)] * WSCALE  # [C, 160]
        out[0:D] = cols.T.reshape(HPC, D, C).transpose(1, 0, 2)
        return np.ascontiguousarray(out.reshape(128, HPC * C)).astype(fp8)

    in_maps = []
    for c in range(NCORES):
        b, hp = divmod(c, NCORES // B)
        qkb = np.zeros((1, 2 * s), np.float32)
        qkb[0, :s] = BROW
        in_maps.append({
            "hsT": hsTs[b],
            "qkb": qkb.astype(fp8),
            "wq": pack_qk(Wq, hp),
            "wk": pack_qk(Wk, hp),
            "wv": pack_v(Wv, hp),
            "wo": pack_wo(Wo, hp),
        })
    return in_maps


def unpermute_o(o_core, s=S):
    """[128, (s/128)*C] partition-major bf16 -> [s, C] f32."""
    nit = s // 128
    return np.asarray(o_core, dtype=np.float32).reshape(
        128, nit, C).transpose(1, 0, 2).reshape(s, C)


def assemble(results, hidden_states, bo, s=S):
    hs = np.asarray(hidden_states, dtype=np.float32)
    bo = np.asarray(bo, dtype=np.float32)
    out = np.empty((B, s, C), dtype=np.float32)
    ncb = NCORES // B
    descale = 1.0 / (WSCALE * WSCALE)
    for b in range(B):
        acc = unpermute_o(results[b * ncb]["o"], s).astype(np.float64)
        for k in range(1, ncb):
            acc = acc + unpermute_o(results[b * ncb + k]["o"], s)
        out[b] = (acc * descale + bo[None, :]).astype(np.float32) + hs[b]
    return out


def kernel(hidden_states, Wq, Wk, Wv, Wo, bo):
    from concourse.bass_utils import run_bass_kernel_spmd

    nc = _get_nc(S)
    in_maps = make_in_maps(hidden_states, Wq, Wk, Wv, Wo)
    res = run_bass_kernel_spmd(nc, in_maps, core_ids=list(range(NCORES)))
    return assemble(res.results, hidden_states, bo)
